# revision 11
# baseline (speedup 1.0000x reference)
"""Grouped self-attention (B=2, S=2048, D=1024, H=16, hd=64) on 8 trn2 cores.

Sharding: core c = b*4 + g handles batch b, heads [4g, 4g+4).

Key simplification: the reference's RoPE indexes its cos/sin cache by the
BATCH dim and uses neg_half = [t_first, -t_second], so rope(t)[b,s,h,d] =
t * (cos(b*th[d%32]) + sign(d)*sin(b*th[d%32])) — a pure per-(b,d) scale
that folds into rows of Wq/Wk on the host. The device kernel is then just
QKV projection + softmax attention.

v4 design (evolved via NTFF profiling):
- all matmul operands bf16 (fp32r streams at ~half rate under throttle),
  PSUM accumulation f32.
- scores as [k part, q free]; exp on ACT with fused 1/8 scale + mask bias;
  phase 2 is ACT-throughput-bound (128 x ~1.1us exp tiles), so everything
  else is organized to keep ACT saturated from ~5us onward.
- K stored as 4 zero-padded per-head tiles ktz[h] [128, S] (real rows in
  the head's pair slot, zeros elsewhere) so scores matmuls run with a
  full 128 contraction (64-contraction matmuls measured ~2.5x slower);
  Q keeps the packed pair layout (padded K rows zero out the other head).
- PV: out[q,d] = et_chunk.T @ V_aug with an appended ones column giving
  the softmax denominator; 4 accumulation slices share a PSUM bank, and
  start=True zeroes a whole bank, so banks are pre-zeroed by DVE memset
  and all PV matmuls accumulate with start=False.
- input x is DMA'd s-major (nb-major pieces) and pair-0 projections are
  emitted per-nb so the first exp fires at ~5us; pair-1 projections are
  interleaved between attention kb iterations of later groups.
- ACT does exp only; all PSUM reads (copies/normalize) are DVE; the
  ktz zero-padding memsets go to GpSimd (SBUF-only engine).

Device layout per core:
  xt    [128, 8*2048] bf16 = x[b].T       (8 k-major chunks, nb-major DMA)
  wqt/wkt/wvt [128, 8*256] bf16           (rope folded into wq/wk rows)
  qt    [2][128, 2048] bf16               (2 head-pairs x 64d rows)
  ktz   [4][128, 2048] bf16               (per-head, zero-padded)
  v_sb  [128, 16, 4, 65] bf16             (s-chunk part, 4 heads, d+ones)
  maskb [128, 16] f32                     ((mask-1)*3e4 bias per kb chunk)
  ost   [128, 16, 256] f32 -> out [2048, 256]
"""

import numpy as np
from contextlib import ExitStack

import ml_dtypes
import concourse.bass as bass
import concourse.bacc as bacc
import concourse.tile as tile
from concourse import mybir
from concourse.bass_utils import run_bass_kernel_spmd

F32 = mybir.dt.float32
BF16 = mybir.dt.bfloat16
EXP = mybir.ActivationFunctionType.Exp

B, S, D, H, HD = 2, 2048, 1024, 16, 64
NCORES = 8

_CACHE = {}


def _build_nc():
    nc = bacc.Bacc("TRN2", target_bir_lowering=False, debug=False)
    # xt_d[nb, kc] = x[b].T[kc*128:(kc+1)*128, nb*512:(nb+1)*512]
    xt_d = nc.declare_dram_parameter("xt", [4, 8, 128, 512], BF16, isOutput=False)
    wqt_d = nc.declare_dram_parameter("wqt", [8, 128, 256], BF16, isOutput=False)
    wkt_d = nc.declare_dram_parameter("wkt", [8, 128, 256], BF16, isOutput=False)
    wvt_d = nc.declare_dram_parameter("wvt", [8, 128, 256], BF16, isOutput=False)
    mb_d = nc.declare_dram_parameter("maskb", [128, 16], F32, isOutput=False)
    out_d = nc.declare_dram_parameter("out", [S, 256], F32, isOutput=True)

    with tile.TileContext(nc) as tc, ExitStack() as ctx:
        const = ctx.enter_context(tc.tile_pool(name="const", bufs=1))
        xpool = ctx.enter_context(tc.tile_pool(name="x", bufs=1))
        wpool = ctx.enter_context(tc.tile_pool(name="w", bufs=1))
        qkpool = ctx.enter_context(tc.tile_pool(name="qk", bufs=1))
        vpool = ctx.enter_context(tc.tile_pool(name="v", bufs=1))
        opool = ctx.enter_context(tc.tile_pool(name="o", bufs=1))
        epool = ctx.enter_context(tc.tile_pool(name="et", bufs=3))
        small = ctx.enter_context(tc.tile_pool(name="small", bufs=4))
        scp = ctx.enter_context(tc.tile_pool(name="scp", bufs=2, space="PSUM"))
        pvp = ctx.enter_context(tc.tile_pool(name="pvp", bufs=2, space="PSUM"))

        # DMA order matters: small/early-needed tensors first, then x
        # nb-major so projections stream against DMA arrival.
        mb = const.tile([128, 16], F32)
        nc.sync.dma_start(mb[:], mb_d[:])
        wk = wpool.tile([128, 8 * 256], BF16, tag="wk")
        wq = wpool.tile([128, 8 * 256], BF16, tag="wq")
        wv = wpool.tile([128, 8 * 256], BF16, tag="wv")
        for wtile, wd in ((wk, wkt_d), (wq, wqt_d), (wv, wvt_d)):
            for c in range(8):
                nc.sync.dma_start(wtile[:, c * 256:(c + 1) * 256], wd[c])
        xt = xpool.tile([128, 8 * S], BF16)
        for nb in range(4):
            for kc in range(8):
                nc.sync.dma_start(
                    xt[:, kc * S + nb * 512: kc * S + nb * 512 + 512],
                    xt_d[nb, kc])

        qt = [qkpool.tile([128, S], BF16, tag=f"qt{p}", name=f"qt{p}")
              for p in range(2)]
        ktz = [qkpool.tile([128, S], BF16, tag=f"ktz{h}", name=f"ktz{h}")
               for h in range(4)]
        # zero the padding rows once (GpSimd: SBUF-only engine, keeps DVE free)
        for h in range(4):
            lo, hi = (64, 128) if h % 2 == 0 else (0, 64)
            nc.gpsimd.memset(ktz[h][lo:hi, :], 0.0)

        v_sb = vpool.tile([128, 16, 4, 65], BF16)
        nc.gpsimd.memset(v_sb[:, :, :, 64:65], 1.0)

        def proj_k(pair, nb):
            ps = scp.tile([128, 1024], F32, tag="sc")
            for kc in range(8):
                lo = kc * 256 + pair * 128
                nc.tensor.matmul(
                    ps[:, 0:512],
                    lhsT=wk[:, lo:lo + 128],
                    rhs=xt[:, kc * S + nb * 512: kc * S + nb * 512 + 512],
                    start=(kc == 0), stop=(kc == 7))
            sl = slice(nb * 512, (nb + 1) * 512)
            nc.vector.tensor_copy(ktz[2 * pair][0:64, sl], ps[0:64, 0:512])
            nc.vector.tensor_copy(ktz[2 * pair + 1][64:128, sl], ps[64:128, 0:512])

        def proj_q(pair, nb):
            ps = scp.tile([128, 1024], F32, tag="sc")
            for kc in range(8):
                lo = kc * 256 + pair * 128
                nc.tensor.matmul(
                    ps[:, 0:512],
                    lhsT=wq[:, lo:lo + 128],
                    rhs=xt[:, kc * S + nb * 512: kc * S + nb * 512 + 512],
                    start=(kc == 0), stop=(kc == 7))
            nc.vector.tensor_copy(qt[pair][:, nb * 512:(nb + 1) * 512],
                                  ps[:, 0:512])

        def proj_v(m):
            pv = scp.tile([128, 1024], F32, tag="sc")
            for kc in range(8):
                nc.tensor.matmul(
                    pv[:, 0:256],
                    lhsT=xt[:, kc * S + m * 128: kc * S + m * 128 + 128],
                    rhs=wv[:, kc * 256:(kc + 1) * 256],
                    start=(kc == 0), stop=(kc == 7))
            nc.vector.tensor_copy(v_sb[:, m, :, 0:64], pv[:, 0:256])

        # ---- phase 1 prefix: pair-0 projections + V, per-nb against DMA ----
        for nb in range(4):
            proj_k(0, nb)
            proj_q(0, nb)
            for m in range(4 * nb, 4 * nb + 4):
                proj_v(m)

        # deferred pair-1 projection pieces, interleaved into phase 2
        deferred = [lambda nb=nb: proj_k(1, nb) for nb in range(4)]
        deferred += [lambda nb=nb: proj_q(1, nb) for nb in range(4)]

        ost = opool.tile([128, 16, 256], F32)

        def attn_group(qh, h, gi):
            pair = h // 2
            # 4 accumulation slices share a PSUM bank and a matmul with
            # start=True zeroes the WHOLE bank, so pre-zero via DVE and
            # accumulate with start=False on every PV matmul.
            pva = pvp.tile([128, 4, 65], F32, tag="pva")
            pvb = pvp.tile([128, 4, 65], F32, tag="pvb")
            nc.vector.memset(pva[:], 0.0)
            nc.vector.memset(pvb[:], 0.0)
            for kb in range(16):
                if gi in (1, 2, 3) and kb in (4, 8, 12) and deferred:
                    deferred.pop(0)()
                ps = scp.tile([128, 1024], F32, tag="sc")
                for j in range(2):
                    q0 = qh * 1024 + j * 512
                    nc.tensor.matmul(
                        ps[:, j * 512:(j + 1) * 512],
                        lhsT=ktz[h][:, kb * 128:(kb + 1) * 128],
                        rhs=qt[pair][:, q0:q0 + 512],
                        start=True, stop=True)
                et = epool.tile([128, 1024], BF16)
                nc.scalar.activation(et[:], ps[:], EXP,
                                     bias=mb[:, kb:kb + 1], scale=0.125)
                for t in range(8):
                    dst = pva if t < 4 else pvb
                    nc.tensor.matmul(
                        dst[:, t % 4, :],
                        lhsT=et[:, t * 128:(t + 1) * 128],
                        rhs=v_sb[:, kb, h, :],
                        start=False, stop=(kb == 15),
                        skip_group_check=True)
            rca = small.tile([128, 4, 1], F32, tag="rca")
            rcb = small.tile([128, 4, 1], F32, tag="rcb")
            nc.vector.reciprocal(rca[:], pva[:, :, 64:65])
            nc.vector.reciprocal(rcb[:], pvb[:, :, 64:65])
            for t in range(8):
                src = pva if t < 4 else pvb
                rc = rca if t < 4 else rcb
                nc.vector.tensor_scalar_mul(
                    ost[:, qh * 8 + t, h * 64:h * 64 + 64],
                    src[:, t % 4, 0:64], rc[:, t % 4, :])

        # ---- phase 2: pair-0 groups (with pair-1 proj interleaved), then
        # pair-1 groups; out rows DMA as soon as a q-half completes all heads
        groups = [(0, 0), (0, 1), (1, 0), (1, 1),
                  (0, 2), (0, 3), (1, 2), (1, 3)]
        for gi, (qh, h) in enumerate(groups):
            attn_group(qh, h, gi)
            if gi == 5:
                for m in range(8):
                    nc.sync.dma_start(out_d[m * 128:(m + 1) * 128, :],
                                      ost[:, m, :])
            if gi == 7:
                for m in range(8, 16):
                    nc.sync.dma_start(out_d[m * 128:(m + 1) * 128, :],
                                      ost[:, m, :])
    nc.compile()
    return nc


def _host_prep(x, attention_mask, Wq, Wk, Wv):
    x = np.asarray(x, dtype=np.float32)
    mask = np.asarray(attention_mask)
    Wq = np.asarray(Wq, dtype=np.float32)
    Wk = np.asarray(Wk, dtype=np.float32)
    Wv = np.asarray(Wv, dtype=np.float32)
    bf16 = ml_dtypes.bfloat16

    # rope fold: c_eff[b, d] = cos(b*th[d%32]) + sign(d)*sin(b*th[d%32])
    j = np.arange(0, HD, 2, dtype=np.float64) / HD          # [32]
    theta = 1.0 / (10000.0 ** j)                            # [32]
    dd = np.arange(HD)
    sign = np.where(dd < 32, 1.0, -1.0)
    in_maps = []
    wvt_full = np.ascontiguousarray(Wv.T).astype(bf16)      # [1024,1024]
    for b in range(B):
        ang = b * theta                                     # [32]
        ce = np.cos(ang[dd % 32]) + sign * np.sin(ang[dd % 32])  # [64]
        ccol = np.tile(ce, H).astype(np.float32)            # [1024]
        wqt_full = np.ascontiguousarray((Wq * ccol[:, None]).T).astype(bf16)
        wkt_full = np.ascontiguousarray((Wk * ccol[:, None]).T).astype(bf16)
        xtT = np.ascontiguousarray(x[b].T).astype(bf16)     # [1024, 2048]
        # [4 nb, 8 kc, 128, 512]
        xt = np.ascontiguousarray(
            xtT.reshape(8, 128, 4, 512).transpose(2, 0, 1, 3))
        maskb = np.ascontiguousarray(
            ((mask[b].astype(np.float32) - 1.0) * 30000.0).reshape(16, 128).T)
        for g in range(4):
            cols = slice(g * 256, (g + 1) * 256)
            in_maps.append({
                "xt": xt,
                "wqt": np.ascontiguousarray(wqt_full[:, cols]).reshape(8, 128, 256),
                "wkt": np.ascontiguousarray(wkt_full[:, cols]).reshape(8, 128, 256),
                "wvt": np.ascontiguousarray(wvt_full[:, cols]).reshape(8, 128, 256),
                "maskb": maskb,
            })
    return in_maps


def _get_nc():
    if "nc" not in _CACHE:
        _CACHE["nc"] = _build_nc()
    return _CACHE["nc"]


def kernel(x, attention_mask, Wq, Wk, Wv, **extra_kwargs):
    nc = _get_nc()
    in_maps = _host_prep(x, attention_mask, Wq, Wk, Wv)
    res = run_bass_kernel_spmd(nc, in_maps, list(range(NCORES))).results
    out = np.empty((B, S, D), dtype=np.float32)
    for c in range(NCORES):
        b, g = divmod(c, 4)
        out[b, :, g * 256:(g + 1) * 256] = res[c]["out"]
    return out


# revision 12
# speedup vs baseline: 1.0530x; 1.0530x over previous
"""Grouped self-attention (B=2, S=2048, D=1024, H=16, hd=64) on 8 trn2 cores.

Sharding: core c = b*4 + g handles batch b, heads [4g, 4g+4).

Key simplification: the reference's RoPE indexes its cos/sin cache by the
BATCH dim and uses neg_half = [t_first, -t_second], so rope(t)[b,s,h,d] =
t * (cos(b*th[d%32]) + sign(d)*sin(b*th[d%32])) — a pure per-(b,d) scale
that folds into rows of Wq/Wk on the host. The device kernel is then just
QKV projection + softmax attention.

v5 design (evolved via NTFF profiling):
- all matmul operands bf16 (fp32r streams at ~half rate), PSUM accum f32.
- phase 2 is ACT-bound: 128 exp tiles [128,1024] at ~1.1us each, ACT
  saturated back-to-back. Everything else aims to start that pipeline
  early and keep it unbroken.
- K stored as 4 zero-padded per-head tiles ktz[h] (real rows in the
  head's pair slot, zeros elsewhere) so scores matmuls run with a full
  128 contraction (64-contraction matmuls measured ~2.5x slower);
  Q keeps the packed pair layout (padded K rows zero the other head).
- PV: out[q,d] = et_chunk.T @ V_aug (ones column = softmax denominator).
  4 accumulation slices share a PSUM bank and start=True zeroes a whole
  bank, so banks are pre-zeroed via DVE memset and PV accumulates with
  start=False.
- DMA uses fat lines only (>=4KB per partition): xt as 8 kc-chunks of
  [128,2048], each weight matrix as a single [128,2048] transfer
  (small-line layouts measured at ~130GB/s vs ~290GB/s).
- the kt0 projection is 4-way chunk-interleaved (4 concurrent PSUM bank
  accumulators) so it tracks DMA arrival and finishes with the last xt
  chunk; first exp fires right after qt0 + V.
- ACT does exp only; PSUM reads (copies/normalize) on DVE; ktz padding
  memsets on GpSimd; output DMA'd per 128-row tile as its last head
  normalizes, hiding the writeback tail.
"""

import numpy as np
from contextlib import ExitStack

import ml_dtypes
import concourse.bass as bass
import concourse.bacc as bacc
import concourse.tile as tile
from concourse import mybir
from concourse.bass_utils import run_bass_kernel_spmd

F32 = mybir.dt.float32
BF16 = mybir.dt.bfloat16
EXP = mybir.ActivationFunctionType.Exp

B, S, D, H, HD = 2, 2048, 1024, 16, 64
NCORES = 8

_CACHE = {}


def _build_nc():
    nc = bacc.Bacc("TRN2", target_bir_lowering=False, debug=False)
    xt_d = nc.declare_dram_parameter("xt", [8, 128, S], BF16, isOutput=False)
    # weights laid out [128, 8, 256] so one fat DMA fills the SBUF tile
    wqt_d = nc.declare_dram_parameter("wqt", [128, 8 * 256], BF16, isOutput=False)
    wkt_d = nc.declare_dram_parameter("wkt", [128, 8 * 256], BF16, isOutput=False)
    wvt_d = nc.declare_dram_parameter("wvt", [128, 8 * 256], BF16, isOutput=False)
    mb_d = nc.declare_dram_parameter("maskb", [128, 16], F32, isOutput=False)
    out_d = nc.declare_dram_parameter("out", [S, 256], F32, isOutput=True)

    with tile.TileContext(nc) as tc, ExitStack() as ctx:
        const = ctx.enter_context(tc.tile_pool(name="const", bufs=1))
        xpool = ctx.enter_context(tc.tile_pool(name="x", bufs=1))
        wpool = ctx.enter_context(tc.tile_pool(name="w", bufs=1))
        qkpool = ctx.enter_context(tc.tile_pool(name="qk", bufs=1))
        vpool = ctx.enter_context(tc.tile_pool(name="v", bufs=1))
        opool = ctx.enter_context(tc.tile_pool(name="o", bufs=1))
        epool = ctx.enter_context(tc.tile_pool(name="et", bufs=3))
        small = ctx.enter_context(tc.tile_pool(name="small", bufs=4))
        scp = ctx.enter_context(tc.tile_pool(name="scp", bufs=2, space="PSUM"))
        pvp = ctx.enter_context(tc.tile_pool(name="pvp", bufs=2, space="PSUM"))

        mb = const.tile([128, 16], F32)
        nc.sync.dma_start(mb[:], mb_d[:])
        wk = wpool.tile([128, 8 * 256], BF16, tag="wk")
        wq = wpool.tile([128, 8 * 256], BF16, tag="wq")
        wv = wpool.tile([128, 8 * 256], BF16, tag="wv")
        nc.sync.dma_start(wk[:], wkt_d[:])
        nc.sync.dma_start(wq[:], wqt_d[:])
        xt = xpool.tile([128, 8 * S], BF16)
        for c in range(8):
            nc.sync.dma_start(xt[:, c * S:(c + 1) * S], xt_d[c])
        nc.sync.dma_start(wv[:], wvt_d[:])

        qt = [qkpool.tile([128, S], BF16, tag=f"qt{p}", name=f"qt{p}")
              for p in range(2)]
        ktz = [qkpool.tile([128, S], BF16, tag=f"ktz{h}", name=f"ktz{h}")
               for h in range(4)]
        # zero the padding rows once (GpSimd: SBUF-only engine, keeps DVE free)
        for h in range(4):
            lo, hi = (64, 128) if h % 2 == 0 else (0, 64)
            nc.gpsimd.memset(ktz[h][lo:hi, :], 0.0)

        v_sb = vpool.tile([128, 16, 4, 65], BF16)
        nc.gpsimd.memset(v_sb[:, :, :, 64:65], 1.0)

        def k_copies(pair, nb, ps):
            sl = slice(nb * 512, (nb + 1) * 512)
            nc.vector.tensor_copy(ktz[2 * pair][0:64, sl], ps[0:64, 0:512])
            nc.vector.tensor_copy(ktz[2 * pair + 1][64:128, sl], ps[64:128, 0:512])

        def proj_q(pair, nb):
            ps = scp.tile([128, 1024], F32, tag="sc")
            for kc in range(8):
                lo = kc * 256 + pair * 128
                nc.tensor.matmul(
                    ps[:, 0:512],
                    lhsT=wq[:, lo:lo + 128],
                    rhs=xt[:, kc * S + nb * 512: kc * S + nb * 512 + 512],
                    start=(kc == 0), stop=(kc == 7))
            nc.vector.tensor_copy(qt[pair][:, nb * 512:(nb + 1) * 512],
                                  ps[:, 0:512])

        def proj_k(pair, nb):
            ps = scp.tile([128, 1024], F32, tag="sc")
            for kc in range(8):
                lo = kc * 256 + pair * 128
                nc.tensor.matmul(
                    ps[:, 0:512],
                    lhsT=wk[:, lo:lo + 128],
                    rhs=xt[:, kc * S + nb * 512: kc * S + nb * 512 + 512],
                    start=(kc == 0), stop=(kc == 7))
            k_copies(pair, nb, ps)

        def proj_v(m):
            pv = scp.tile([128, 1024], F32, tag="sc")
            for kc in range(8):
                nc.tensor.matmul(
                    pv[:, 0:256],
                    lhsT=xt[:, kc * S + m * 128: kc * S + m * 128 + 128],
                    rhs=wv[:, kc * 256:(kc + 1) * 256],
                    start=(kc == 0), stop=(kc == 7))
            nc.vector.tensor_copy(v_sb[:, m, :, 0:64], pv[:, 0:256])

        # ---- phase 1 ----
        # kt0: 4 nb-groups chunk-interleaved across 4 PSUM banks (2 sc
        # tiles x 2 bank-halves) so the projection tracks xt DMA arrival.
        t1 = scp.tile([128, 1024], F32, tag="sc")
        t2 = scp.tile([128, 1024], F32, tag="sc")
        slots = [t1[:, 0:512], t1[:, 512:1024], t2[:, 0:512], t2[:, 512:1024]]
        for kc in range(8):
            for nb in range(4):
                nc.tensor.matmul(
                    slots[nb],
                    lhsT=wk[:, kc * 256: kc * 256 + 128],
                    rhs=xt[:, kc * S + nb * 512: kc * S + nb * 512 + 512],
                    start=(kc == 0), stop=(kc == 7))
        for nb in range(4):
            sl = slice(nb * 512, (nb + 1) * 512)
            nc.vector.tensor_copy(ktz[0][0:64, sl], slots[nb][0:64, :])
            nc.vector.tensor_copy(ktz[1][64:128, sl], slots[nb][64:128, :])
        for nb in range(4):
            proj_q(0, nb)
        for m in range(16):
            proj_v(m)
        for nb in range(4):
            proj_k(1, nb)
        for nb in range(4):
            proj_q(1, nb)

        # ---- phase 2 ----
        ost = opool.tile([128, 16, 256], F32)

        def attn_group(qh, h, last_head):
            pair = h // 2
            # 4 accumulation slices share a PSUM bank and a matmul with
            # start=True zeroes the WHOLE bank, so pre-zero via DVE and
            # accumulate with start=False on every PV matmul.
            pva = pvp.tile([128, 4, 65], F32, tag="pva")
            pvb = pvp.tile([128, 4, 65], F32, tag="pvb")
            nc.vector.memset(pva[:], 0.0)
            nc.vector.memset(pvb[:], 0.0)
            for kb in range(16):
                ps = scp.tile([128, 1024], F32, tag="sc")
                for j in range(2):
                    q0 = qh * 1024 + j * 512
                    nc.tensor.matmul(
                        ps[:, j * 512:(j + 1) * 512],
                        lhsT=ktz[h][:, kb * 128:(kb + 1) * 128],
                        rhs=qt[pair][:, q0:q0 + 512],
                        start=True, stop=True)
                et = epool.tile([128, 1024], BF16)
                nc.scalar.activation(et[:], ps[:], EXP,
                                     bias=mb[:, kb:kb + 1], scale=0.125)
                for t in range(8):
                    dst = pva if t < 4 else pvb
                    nc.tensor.matmul(
                        dst[:, t % 4, :],
                        lhsT=et[:, t * 128:(t + 1) * 128],
                        rhs=v_sb[:, kb, h, :],
                        start=False, stop=(kb == 15),
                        skip_group_check=True)
            rca = small.tile([128, 4, 1], F32, tag="rca")
            rcb = small.tile([128, 4, 1], F32, tag="rcb")
            nc.vector.reciprocal(rca[:], pva[:, :, 64:65])
            nc.vector.reciprocal(rcb[:], pvb[:, :, 64:65])
            for t in range(8):
                src = pva if t < 4 else pvb
                rc = rca if t < 4 else rcb
                m = qh * 8 + t
                nc.vector.tensor_scalar_mul(
                    ost[:, m, h * 64:h * 64 + 64],
                    src[:, t % 4, 0:64], rc[:, t % 4, :])
                if last_head:
                    nc.sync.dma_start(out_d[m * 128:(m + 1) * 128, :],
                                      ost[:, m, :])

        for qh in range(2):
            for h in range(4):
                attn_group(qh, h, last_head=(h == 3))
    nc.compile()
    return nc


def _host_prep(x, attention_mask, Wq, Wk, Wv):
    x = np.asarray(x, dtype=np.float32)
    mask = np.asarray(attention_mask)
    Wq = np.asarray(Wq, dtype=np.float32)
    Wk = np.asarray(Wk, dtype=np.float32)
    Wv = np.asarray(Wv, dtype=np.float32)
    bf16 = ml_dtypes.bfloat16

    # rope fold: c_eff[b, d] = cos(b*th[d%32]) + sign(d)*sin(b*th[d%32])
    j = np.arange(0, HD, 2, dtype=np.float64) / HD          # [32]
    theta = 1.0 / (10000.0 ** j)                            # [32]
    dd = np.arange(HD)
    sign = np.where(dd < 32, 1.0, -1.0)

    def wlayout(wt_cols):  # [1024(k), 256] -> [128, 8*256] (partition-major)
        return np.ascontiguousarray(
            wt_cols.reshape(8, 128, 256).transpose(1, 0, 2).reshape(128, 8 * 256))

    in_maps = []
    wvt_full = np.ascontiguousarray(Wv.T).astype(bf16)      # [1024,1024]
    for b in range(B):
        ang = b * theta                                     # [32]
        ce = np.cos(ang[dd % 32]) + sign * np.sin(ang[dd % 32])  # [64]
        ccol = np.tile(ce, H).astype(np.float32)            # [1024]
        wqt_full = np.ascontiguousarray((Wq * ccol[:, None]).T).astype(bf16)
        wkt_full = np.ascontiguousarray((Wk * ccol[:, None]).T).astype(bf16)
        xt = np.ascontiguousarray(x[b].T).astype(bf16).reshape(8, 128, S)
        maskb = np.ascontiguousarray(
            ((mask[b].astype(np.float32) - 1.0) * 30000.0).reshape(16, 128).T)
        for g in range(4):
            cols = slice(g * 256, (g + 1) * 256)
            in_maps.append({
                "xt": xt,
                "wqt": wlayout(wqt_full[:, cols]),
                "wkt": wlayout(wkt_full[:, cols]),
                "wvt": wlayout(wvt_full[:, cols]),
                "maskb": maskb,
            })
    return in_maps


def _get_nc():
    if "nc" not in _CACHE:
        _CACHE["nc"] = _build_nc()
    return _CACHE["nc"]


def kernel(x, attention_mask, Wq, Wk, Wv, **extra_kwargs):
    nc = _get_nc()
    in_maps = _host_prep(x, attention_mask, Wq, Wk, Wv)
    res = run_bass_kernel_spmd(nc, in_maps, list(range(NCORES))).results
    out = np.empty((B, S, D), dtype=np.float32)
    for c in range(NCORES):
        b, g = divmod(c, 4)
        out[b, :, g * 256:(g + 1) * 256] = res[c]["out"]
    return out
